# revision 37
# baseline (speedup 1.0000x reference)
"""Trainium2 Bass kernel for nn_MultiHeadBindingAttention.

Reference computation (B=4, T=2048, D=4096, H=4, HD=1024):
    q_bind = alpha_q * sign(bv_q)   (per head; zeros -> +alpha)
    Q = xh * q_bind ; K = xh * k_bind ; V = xh * v_bind
    scores = einsum('bthd,bshd->bhts', Q, K) / sqrt(HD)
    attn   = where(causal, sigmoid(4*scores), 0)
    out    = einsum('bhts,bshd->bthd', attn, V)

Algebraic restructuring:
    sigmoid argument  z = c_h * M[t,s],  M = x sgn_qk x^T,
        c_h = 4 * alpha_q[h] * alpha_k[h] / sqrt(HD)
    attn = 0.5 * causal_mask + R,   R = 0.5 * tanh(z/2)   (exact identity)
    out[t] = 0.5 * sum_{s<=t} xv[s]  +  sum_{s<=t} R[t,s] * xv[s]
        xv[s,d] = x[s,d] * v_bind[h,d]
    The first term is a prefix sum of xv — O(T*HD), computed exactly in
    f32 on the host and added during output assembly (HOST_PF). The second
    term runs entirely in fp8 DoubleRow on the tensor engine: tanh tiles
    are written directly to fp8e5 by the scalar engine (the 0.5 factor is
    folded into the fp8 copy of xv, c_h/2 into the activation input scale).
    Both matmul phases use DoubleRowSwInterleave (stationary operands are
    pre-interleaved, by the host for scores and by the activation's strided
    output AP for AV) so weight loads read contiguously, and diagonal score
    tiles skip the columns left of the causal diagonal.

Sharding: the 16 (b,h) pairs are data-parallel; each of the 8 cores gets 2.
Scores are computed in [s,t] orientation (M is symmetric), so the tanh tile
is already transposed for the R^T @ xv matmul.
"""

import numpy as np

import concourse.bacc as bacc
import concourse.tile as tile
from concourse import mybir
from concourse.bass_utils import run_bass_kernel_spmd

B, T, D = 4, 2048, 4096
H, HD = 4, 1024
N_CORES = 8
PAIRS = 2                      # (b,h) pairs per core
P = 128                        # partitions
TB = 512                       # t-block (strip) width
NTB = T // TB                  # 4 strips
DRCH = HD // (2 * P)           # 4 double-row contraction chunks of 256
NSP = T // (2 * P)             # 8 double-row s-pair chunks

F32 = mybir.dt.float32
F16 = mybir.dt.float16
SC_DT = mybir.dt.float8e4      # scores operands
AV_DT = mybir.dt.float8e5      # R / xv operands (values ~1e-3 need e5 range)

import os
SW_SCORES = os.environ.get("SW_SCORES", "1") == "1"
SW_AV = os.environ.get("SW_AV", "1") == "1"
SPLIT_CP = os.environ.get("SPLIT_CP", "0") == "1"
HOST_PF = os.environ.get("HOST_PF", "1") == "1"
NSC = T // P                   # 16 s-chunks


CP_ACT = os.environ.get("CP_ACT", "1") == "1"
PS3 = os.environ.get("PS3", "0") == "1"
AV_INSTRIP = os.environ.get("AV_INSTRIP", "1") == "1"
LIN_DVE = os.environ.get("LIN_DVE", "0") == "1"


def set_variant(**kw):
    """Override variant flags."""
    g = globals()
    for k, v in kw.items():
        assert k in ("SW_SCORES", "SW_AV", "SPLIT_CP", "HOST_PF",
                     "CP_ACT", "PS3", "AV_INSTRIP", "LIN_DVE"), k
        g[k] = v

_program_cache = None


def _build_program(reps=1, phase="full"):
    nc = bacc.Bacc(
        trn_type="TRN2", target_bir_lowering=False, debug=False,
        num_devices=N_CORES,
    )
    if SW_SCORES:
        xst_ap = nc.dram_tensor(
            "xst", [PAIRS, DRCH, P, NSC, 2, P], SC_DT,
            kind="ExternalInput").ap()
    else:
        xst_ap = nc.dram_tensor(
            "xst", [PAIRS, DRCH, P, 2, T], SC_DT, kind="ExternalInput").ap()
    xpt_ap = nc.dram_tensor(
        "xpt", [PAIRS, DRCH, P, 2, T], SC_DT, kind="ExternalInput").ap()
    xv8_ap = nc.dram_tensor(
        "xv8", [PAIRS, NSP, P, 2, HD], AV_DT, kind="ExternalInput").ap()
    pf_ap = None
    if not HOST_PF:
        pf_ap = nc.dram_tensor(
            "pf", [PAIRS, T, HD], F16, kind="ExternalInput").ap()
    cvec_ap = nc.dram_tensor("cvec", [PAIRS, P, 1], F32, kind="ExternalInput").ap()
    out_ap = nc.dram_tensor("out", [PAIRS, T, HD], F16, kind="ExternalOutput").ap()

    with tile.TileContext(nc) as tc:
        with (
            tc.tile_pool(name="xst", bufs=2 * DRCH) as xst_pool,
            tc.tile_pool(name="xpt", bufs=2 * DRCH) as xpt_pool,
            tc.tile_pool(name="xv8", bufs=NSP + 2) as xv8_pool,
            tc.tile_pool(name="pf", bufs=18) as pf_pool,
            tc.tile_pool(name="a8", bufs=22) as a8_pool,
            tc.tile_pool(name="outsb", bufs=6) as out_pool,
            tc.tile_pool(name="tmp16", bufs=4) as tmp_pool,
            tc.tile_pool(name="cvec", bufs=PAIRS) as c_pool,
            tc.tile_pool(name="psum_s", bufs=3 if PS3 else 2,
                         space="PSUM") as ps_pool,
            tc.tile_pool(name="psum_o", bufs=5 if PS3 else 6,
                         space="PSUM") as po_pool,
        ):
            for bh in [bh for _ in range(reps) for bh in range(PAIRS)]:
                # ---- load inputs for this (b,h) ----
                # xst/xpt on the sync HWDGE ring; xv8/pf on the gpsimd
                # SWDGE ring so they don't queue behind them; out stores
                # go out on the scalar HWDGE ring.
                cvec_t = c_pool.tile([P, 1], F32)
                nc.sync.dma_start(cvec_t[:], cvec_ap[bh])
                xst_t = []
                xpt_t = []
                if phase != "av":
                    for k in range(DRCH):
                        if SW_SCORES:
                            t1 = xst_pool.tile([P, NSC, 2, P], SC_DT)
                        else:
                            t1 = xst_pool.tile([P, 2, T], SC_DT)
                        nc.sync.dma_start(t1[:], xst_ap[bh, k])
                        xst_t.append(t1)
                        t2 = xpt_pool.tile([P, 2, T], SC_DT)
                        nc.sync.dma_start(t2[:], xpt_ap[bh, k])
                        xpt_t.append(t2)
                xv8_t = []
                pf_t = []
                if phase != "scores":
                    for r in range(NSP):
                        t3 = xv8_pool.tile([P, 2, HD], AV_DT)
                        nc.gpsimd.dma_start(t3[:], xv8_ap[bh, r])
                        xv8_t.append(t3)
                    if not HOST_PF:
                        for i in range(T // P):
                            t4 = pf_pool.tile([P, HD], F16)
                            nc.sync.dma_start(
                                t4[:], pf_ap[bh, i * P:(i + 1) * P, :])
                            pf_t.append(t4)

                a8_t = [[] for _ in range(NTB)]

                def scores_chunk(j, c):
                    """R[s,t] tile: s chunk c, t in [TB*j, TB*(j+1)),
                    written as fp8e5 into double-row pair tile slot c%2.

                    Diagonal chunks (c*P >= t0) skip the t < c*P columns:
                    those are entirely above the causal diagonal, and the
                    full-width affine_select zero-fills them (the skipped
                    region of ps/a8 is never read except through the mask).
                    """
                    t0 = TB * j
                    off = max(0, c * P - t0)
                    ps = ps_pool.tile([P, TB], F32, name=f"ps_{bh}_{j}_{c}",
                                      tag="ps")
                    for k in range(DRCH):
                        if SW_SCORES:
                            nc.tensor.matmul(
                                ps[:, off:],
                                xst_t[k][:, c, :, :],
                                xpt_t[k][:, :, t0 + off:t0 + TB],
                                start=(k == 0), stop=(k == DRCH - 1),
                                perf_mode=mybir.MatmulPerfMode.DoubleRowSwInterleave,
                            )
                        else:
                            nc.tensor.matmul(
                                ps[:, off:],
                                xst_t[k][:, :, c * P:(c + 1) * P],
                                xpt_t[k][:, :, t0 + off:t0 + TB],
                                start=(k == 0), stop=(k == DRCH - 1),
                                perf_mode=mybir.MatmulPerfMode.DoubleRow,
                            )
                    p2, slot = divmod(c, 2)
                    if slot == 0:
                        shape = [P, TB // P, P, 2] if SW_AV else [P, 2, TB]
                        a8_t[j].append(a8_pool.tile(
                            shape, AV_DT,
                            name=f"a8_{bh}_{j}_{p2}", tag="a8"))
                    a8 = a8_t[j][p2]
                    if SW_AV:
                        b0 = off // P
                        if LIN_DVE and c % 2 == 0:
                            # |z/2| < 3e-3 here, so tanh(z/2) = z/2 to ~1e-5
                            # relative -- far below the fp8e5 quantization of
                            # this tile. Offload to the otherwise-idle DVE.
                            nc.vector.tensor_scalar(
                                a8[:, b0:, ::-1, slot], ps[:, off:],
                                cvec_t[:], 0.5,
                                op0=mybir.AluOpType.mult,
                                op1=mybir.AluOpType.mult,
                            )
                        else:
                            nc.scalar.activation(
                                a8[:, b0:, ::-1, slot], ps[:, off:],
                                mybir.ActivationFunctionType.Tanh,
                                scale=cvec_t[:],
                            )
                        if c * P >= t0:  # diagonal tile: zero where t < s
                            nc.gpsimd.affine_select(
                                out=a8[:, :, ::-1, slot],
                                in_=a8[:, :, ::-1, slot],
                                compare_op=mybir.AluOpType.is_ge,
                                fill=0.0,
                                base=t0 - c * P,
                                pattern=[[P, TB // P], [1, P]],
                                channel_multiplier=-1,
                            )
                    else:
                        nc.scalar.activation(
                            a8[:, slot, off:], ps[:, off:],
                            mybir.ActivationFunctionType.Tanh,
                            scale=cvec_t[:],
                        )
                        if c * P >= t0:  # diagonal tile: zero where t < s
                            nc.gpsimd.affine_select(
                                out=a8[:, slot, :], in_=a8[:, slot, :],
                                compare_op=mybir.AluOpType.is_ge,
                                fill=0.0,
                                base=t0 - c * P,
                                pattern=[[1, TB]],
                                channel_multiplier=-1,
                            )

                def av_i(j, i):
                    """out rows [128i, 128i+128) from strip j's R tiles."""
                    toff = i * P - TB * j
                    npair = i // 2 + 1
                    osb = out_pool.tile([P, HD], F16)
                    po = [po_pool.tile([P, TB], F32,
                                       name=f"po_{bh}_{i}_{h2}", tag="po")
                          for h2 in range(2)]
                    for p2 in range(npair):
                        if SW_AV:
                            st = a8_t[j][p2][:, toff // P, :, :]
                            pm = mybir.MatmulPerfMode.DoubleRowSwInterleave
                        else:
                            st = a8_t[j][p2][:, :, toff:toff + P]
                            pm = mybir.MatmulPerfMode.DoubleRow
                        for half in range(2):
                            nc.tensor.matmul(
                                po[half][:],
                                st,
                                xv8_t[p2][:, :, half * TB:(half + 1) * TB],
                                start=(p2 == 0), stop=(p2 == npair - 1),
                                perf_mode=pm,
                            )
                    if HOST_PF:
                        # pf is added on the host; device only moves the
                        # R-term psum out (CP_ACT: both halves on the scalar
                        # engine, leaving the vector engine idle)
                        if CP_ACT:
                            nc.scalar.activation(
                                osb[:, 0:TB], po[0][:],
                                mybir.ActivationFunctionType.Copy,
                            )
                        else:
                            nc.vector.tensor_copy(osb[:, 0:TB], po[0][:])
                        nc.scalar.activation(
                            osb[:, TB:2 * TB], po[1][:],
                            mybir.ActivationFunctionType.Copy,
                        )
                        nc.scalar.dma_start(
                            out_ap[bh, i * P:(i + 1) * P, :], osb[:])
                        return
                    nc.vector.tensor_tensor(
                        osb[:, 0:TB],
                        po[0][:],
                        pf_t[i][:, 0:TB],
                        op=mybir.AluOpType.add,
                    )
                    if SPLIT_CP:
                        # route half 1 psum->sbuf through the scalar engine
                        # (underloaded), then a cheap 2x-mode fp16 DVE add
                        tmp = tmp_pool.tile([P, TB], F16,
                                            name=f"tmp_{bh}_{i}", tag="tmp")
                        nc.scalar.activation(
                            tmp[:], po[1][:],
                            mybir.ActivationFunctionType.Copy,
                        )
                        nc.vector.tensor_tensor(
                            osb[:, TB:2 * TB],
                            tmp[:],
                            pf_t[i][:, TB:2 * TB],
                            op=mybir.AluOpType.add,
                        )
                    else:
                        nc.vector.tensor_tensor(
                            osb[:, TB:2 * TB],
                            po[1][:],
                            pf_t[i][:, TB:2 * TB],
                            op=mybir.AluOpType.add,
                        )
                    nc.scalar.dma_start(out_ap[bh, i * P:(i + 1) * P, :], osb[:])

                # fine-grained software pipeline: strip j's score chunks are
                # interleaved with strip j-1's AV row-chunks so the PE queue
                # mixes both matmul streams and DVE/ACT work spreads out.
                if phase == "dma":
                    for i in range(T // P):
                        osb = out_pool.tile([P, HD], F16)
                        nc.vector.memset(osb[:], 0.0)
                        nc.scalar.dma_start(
                            out_ap[bh, i * P:(i + 1) * P, :], osb[:])
                elif phase == "scores":
                    for j in range(NTB):
                        for c in range(4 * (j + 1)):
                            scores_chunk(j, c)
                elif phase == "av":
                    for j in range(NTB):
                        for p2 in range(2 * j + 2):
                            shape = ([P, TB // P, P, 2] if SW_AV
                                     else [P, 2, TB])
                            a8_t[j].append(a8_pool.tile(
                                shape, AV_DT,
                                name=f"a8_{bh}_{j}_{p2}", tag="a8"))
                            nc.vector.memset(a8_t[j][p2][:], 0.25)
                        for i in range(4 * j, 4 * j + 4):
                            av_i(j, i)
                elif AV_INSTRIP:
                    # emit each AV row-chunk as soon as its own strip's
                    # dependency chunk is done (shorter tail, tighter deps)
                    for j in range(NTB):
                        for c in range(4 * (j + 1)):
                            scores_chunk(j, c)
                            if c == 4 * j + 1:
                                av_i(j, 4 * j)
                                av_i(j, 4 * j + 1)
                            elif c == 4 * j + 3:
                                av_i(j, 4 * j + 2)
                                av_i(j, 4 * j + 3)
                else:
                    for j in range(NTB):
                        avis = list(range(4 * (j - 1), 4 * j)) if j > 0 else []
                        ai = 0
                        for c in range(4 * (j + 1)):
                            scores_chunk(j, c)
                            if ai < len(avis) and c % 2 == 1:
                                av_i(j - 1, avis[ai])
                                ai += 1
                        while ai < len(avis):
                            av_i(j - 1, avis[ai])
                            ai += 1
                    for i in range(4 * (NTB - 1), 4 * NTB):
                        av_i(NTB - 1, i)

    nc.compile()
    return nc


def get_program():
    global _program_cache
    if _program_cache is None:
        _program_cache = _build_program()
    return _program_cache


def _sign_pm1(w):
    s = np.sign(w)
    return np.where(s == 0, 1.0, s).astype(np.float32)


def make_in_maps(x, bv_q, bv_k, bv_v):
    x = np.asarray(x, dtype=np.float32)
    bv_q = np.asarray(bv_q, dtype=np.float32)
    bv_k = np.asarray(bv_k, dtype=np.float32)
    bv_v = np.asarray(bv_v, dtype=np.float32)

    alpha_q = np.abs(bv_q).mean(axis=-1)          # [H]
    alpha_k = np.abs(bv_k).mean(axis=-1)
    alpha_v = np.abs(bv_v).mean(axis=-1)
    sgn_qk = _sign_pm1(bv_q) * _sign_pm1(bv_k)    # [H, HD]
    v_bind = alpha_v[:, None] * _sign_pm1(bv_v)   # [H, HD]
    c = (4.0 * (HD ** -0.5)) * alpha_q * alpha_k  # [H]

    import ml_dtypes
    FP8S = ml_dtypes.float8_e4m3fn
    FP8A = ml_dtypes.float8_e5m2

    xh = x.reshape(B, T, H, HD)
    in_maps = []
    for core in range(N_CORES):
        if SW_SCORES:
            xst = np.empty((PAIRS, DRCH, P, NSC, 2, P), FP8S)
        else:
            xst = np.empty((PAIRS, DRCH, P, 2, T), FP8S)
        xpt = np.empty((PAIRS, DRCH, P, 2, T), FP8S)
        xv8 = np.empty((PAIRS, NSP, P, 2, HD), FP8A)
        pf = np.empty((PAIRS, T, HD), np.float16)
        cvec = np.empty((PAIRS, P, 1), np.float32)
        for slot in range(PAIRS):
            bh = PAIRS * core + slot
            b, h = divmod(bh, H)
            xs = xh[b, :, h, :]                      # [T, HD] f32
            xsT = np.ascontiguousarray(xs.T)         # [HD, T]
            xss = xsT * sgn_qk[h][:, None]
            if SW_SCORES:
                # SW-interleaved stationary: per (k, p, c) a contiguous
                # 256-elem block [A_127, B_127, ..., A_0, B_0] with
                # A_m = xss[256k+p, 128c+m], B_m = xss[256k+128+p, 128c+m]
                A = xss.reshape(DRCH, 2, P, NSC, P)      # [k, i, p, c, m]
                xst[slot] = np.ascontiguousarray(
                    A.transpose(0, 2, 3, 4, 1)[:, :, :, ::-1, :]
                ).reshape(DRCH, P, NSC, 2, P).astype(FP8S)
            else:
                # pair layout [r, p, i, t] with d = 256r + 128i + p
                xst[slot] = xss.reshape(
                    DRCH, 2, P, T).transpose(0, 2, 1, 3).astype(FP8S)
            xpt[slot] = xsT.reshape(
                DRCH, 2, P, T).transpose(0, 2, 1, 3).astype(FP8S)
            xv = xs * v_bind[h][None, :]             # [T, HD] f32
            # pair layout [r, p, i, d] with s = 256r + 128i + p
            xv8[slot] = (0.5 * xv).reshape(
                NSP, 2, P, HD).transpose(0, 2, 1, 3).astype(FP8A)
            if not HOST_PF:
                pf[slot] = (0.5 * np.cumsum(xv, axis=0)).astype(np.float16)
            cvec[slot] = c[h] / 2.0
        m = {"xst": xst, "xpt": xpt, "xv8": xv8, "cvec": cvec}
        if not HOST_PF:
            m["pf"] = pf
        in_maps.append(m)
    return in_maps


def make_host_pf(x, bv_v):
    """0.5 * causal prefix sums of xv, computed exactly in f32 on the host."""
    x = np.asarray(x, dtype=np.float32)
    bv_v = np.asarray(bv_v, dtype=np.float32)
    alpha_v = np.abs(bv_v).mean(axis=-1)
    v_bind = alpha_v[:, None] * _sign_pm1(bv_v)
    xh = x.reshape(B, T, H, HD)
    pf = np.empty((B, H, T, HD), np.float32)
    for b in range(B):
        for h in range(H):
            xv = xh[b, :, h, :] * v_bind[h][None, :]
            pf[b, h] = 0.5 * np.cumsum(xv, axis=0)
    return pf


def assemble_output(results, pf_host=None):
    out = np.empty((B, T, D), np.float32)
    oh = out.reshape(B, T, H, HD)
    for core in range(N_CORES):
        for slot in range(PAIRS):
            bh = PAIRS * core + slot
            b, h = divmod(bh, H)
            r = results[core]["out"][slot].astype(np.float32)
            if pf_host is not None:
                r = r + pf_host[b, h]
            oh[b, :, h, :] = r
    return out


def kernel(x, bv_q, bv_k, bv_v):
    nc = get_program()
    in_maps = make_in_maps(x, bv_q, bv_k, bv_v)
    pf_host = make_host_pf(x, bv_v) if HOST_PF else None
    res = run_bass_kernel_spmd(nc, in_maps, list(range(N_CORES)))
    return assemble_output(res.results, pf_host)
